# revision 45
# baseline (speedup 1.0000x reference)
"""Fused multi-head attention on 8 TRN2 NeuronCores.

Problem: x[2,2048,1024] -> q,k,v = x@W.T+b (16 heads x 64), softmax(q k^T/8) v,
then out @ Wp.T + bp.

Sharding: data-parallel over batch (2) x tensor-parallel over heads (4 ranks x
4 heads = 256 dims, Megatron-style).  Core c handles batch c//4, head-rank c%4.
The proj partial sums are reduced on the host (numpy), and the v-bias and
proj-bias are folded into one host-side vector bp_eff = bv @ Wp.T + bp.

Per-core layouts (host pre-transposes/pre-tiles, all DMA rows are >=512B
contiguous DRAM runs):
  xT  [1024, 2048]  x[b].T
  wqT/wkT/wvT [128, 8*256]  W.T slice pre-tiled so partition p holds all 8
                            contraction tiles contiguously
  wpT [256, 1024]           Wp.T rows for this rank's 256 dims
  bq/bk [256, 1]
  outT [1024, 2048] bf16 partial (x[b] @ ..).T, missing bv/bp contributions

Schedule (all matmul operands bfloat16, fp32 PSUM accumulate):
  head:   x arrives as 4x 1MB DMAs (2 k-tiles each).  The k/q mt=0
          projections accumulate kt-major across all 8 PSUM banks, paced
          by x arrival, so the DMA window is PE-busy.  Biases are applied
          psum->SBUF with ScalarE activation(Identity, bias) and DVE
          tensor_scalar_add in parallel, k-chunk0/q-chunk0 first so the
          attention stream starts immediately after the last x tile.
  stream: ONE flat software-pipelined stream over all 128 (n-chunk,
          head-pair, key-block) blocks; per block:
            sT[m, n] = kT.T @ qT   two heads row-packed (concurrent K=64
                                   matmuls in distinct PE row groups)
            p = exp(sT / 8)        ACT, one [128,1024] instr, both heads
            po_h[:, n] += vpk[mb,h].T @ p_h  per head: psum partitions
                                   0:64 = P@V, 64:128 = softmax denominator
                                   (ones-columns of vpk) -- no den matmuls.
          PV trails its block's exp by one position globally.  The stream
          is ACT-paced (~1.16us/block); fillers pumped per block absorb
          the PE slack: v-projection (position 0, JIT per key block),
          k/q mt=1 projections with chunk-interleaved lhsT reuse
          (positions 1-2), output projection chunks 0-2 (positions 5-7).
  attnT = po[0:64] * reciprocal_approx_fast(po[64:128]) per head
  tail:   chunk-3 outproj runs as 8 K=256 psum-accumulated groups (no
          SBUF staging / no DVE adds); psum->SBUF copies alternate
          Vector/Scalar, output DMAs round-robin 3 engines; the K-half
          reading at(3,0) starts during the final normalize.
"""

import numpy as np

DIM = 1024
N_TOK = 2048
N_HEADS_LOC = 4       # heads per core
D_LOC = 256           # local q/k/v dims per core
SCALE = 64 ** -0.5
P = 128
CH = 512              # n-chunk (moving free dim)
NCH = N_TOK // CH     # 4
KT = DIM // P         # 8 contraction tiles for qkv/proj
MB = N_TOK // P       # 16 key blocks
N_CORES = 8

_NC_CACHE = {}


def build_nc(dt_mm_name="bfloat16"):
    import concourse.mybir as mybir
    import concourse.tile as tile
    from concourse import bacc
    from concourse.bass import ts

    f32 = mybir.dt.float32
    dt_mm = getattr(mybir.dt, dt_mm_name)
    Exp = mybir.ActivationFunctionType.Exp

    nc = bacc.Bacc("TRN2", target_bir_lowering=False, debug=False,
                   num_devices=N_CORES)
    xT = nc.dram_tensor("xT", [DIM, N_TOK], dt_mm, kind="ExternalInput").ap()
    wqT = nc.dram_tensor("wqT", [P, KT * D_LOC], dt_mm, kind="ExternalInput").ap()
    wkT = nc.dram_tensor("wkT", [P, KT * D_LOC], dt_mm, kind="ExternalInput").ap()
    wvT = nc.dram_tensor("wvT", [P, KT * D_LOC], dt_mm, kind="ExternalInput").ap()
    wpT = nc.dram_tensor("wpT", [D_LOC, DIM], dt_mm, kind="ExternalInput").ap()
    bq = nc.dram_tensor("bq", [D_LOC, 1], f32, kind="ExternalInput").ap()
    bk = nc.dram_tensor("bk", [D_LOC, 1], f32, kind="ExternalInput").ap()
    outT = nc.dram_tensor("outT", [DIM, N_TOK], dt_mm, kind="ExternalOutput").ap()
    wsink_dram = nc.dram_tensor("wsink", [2, 8], f32, kind="ExternalOutput").ap()

    with tile.TileContext(nc) as tc:
        with (
            tc.tile_pool(name="const", bufs=1) as const,
            tc.tile_pool(name="work", bufs=2) as work,
            # scores: 2x [128,1024] = 4 banks; also pass-1 k-mt0 accums
            tc.tile_pool(name="ps_s", bufs=2, space="PSUM") as ps_s,
            # filler ring (projections, vproj, outproj): 2x [128,512]
            tc.tile_pool(name="ps_pj", bufs=2, space="PSUM") as ps_pj,
            # PV+den accumulators, one per head of the active pair: 2 banks
            tc.tile_pool(name="ps_po", bufs=1, space="PSUM") as ps_po,
        ):
            # ---- persistent SBUF state ----
            w_tiles = {}
            for name in ("k", "q", "v"):
                w_tiles[name] = const.tile([P, KT, D_LOC], dt_mm, tag=f"w{name}",
                                           name=f"w{name}")
            x_sb = const.tile([P, KT, N_TOK], dt_mm, tag="x", name="x")

            # DMA issue order: each engine's dma_starts serialize (~0.65us
            # each) and the queues round-robin in-flight transfers at packet
            # granularity, so total wire time (~6.3MB at ~300GB/s = 21us)
            # bounds the head.  x goes as 4x 1MB transfers (128-partition,
            # >=1MB is the max-bandwidth shape); wv is issued first on the
            # vector queue so the position-0 v-projection isn't gated at
            # stream start; wp (needed at position 3) queues last.
            def w_slice(dst, src_ap, k0, k1):
                nc.sync.dma_start(
                    out=dst[:, k0:k1, :],
                    in_=src_ap[:, k0 * D_LOC:k1 * D_LOC].rearrange(
                        "p (k n) -> p k n", k=k1 - k0))

            # x: kt0 and kt1 as 512KB transfers (kt0 gates the first pass-1
            # matmuls), the rest as 1MB; wv rides the END of the (short)
            # sync queue so it lands ~23us, before the position-0
            # v-projection needs it; queue loads balance to ~2-2.5MB each.
            def x_slice(k0, k1):
                return xT[k0 * P:k1 * P, :].rearrange(
                    "(k p) n -> p k n", k=k1 - k0)
            nc.gpsimd.dma_start(out=x_sb[:, 0:1, :], in_=x_slice(0, 1))
            nc.scalar.dma_start(out=x_sb[:, 2:4, :], in_=x_slice(2, 4))
            nc.gpsimd.dma_start(out=x_sb[:, 1:2, :], in_=x_slice(1, 2))
            w_slice(w_tiles["k"], wkT, 0, 2)
            w_slice(w_tiles["q"], wqT, 0, 2)
            nc.gpsimd.dma_start(out=x_sb[:, 4:6, :], in_=x_slice(4, 6))
            nc.scalar.dma_start(out=x_sb[:, 6:8, :], in_=x_slice(6, 8))
            w_slice(w_tiles["k"], wkT, 2, 4)
            w_slice(w_tiles["q"], wqT, 2, 4)
            w_slice(w_tiles["k"], wkT, 4, 6)
            w_slice(w_tiles["q"], wqT, 4, 6)
            w_slice(w_tiles["k"], wkT, 6, 8)
            w_slice(w_tiles["q"], wqT, 6, 8)
            bias_sb = {}
            for name, src_ap in (("q", bq), ("k", bk)):
                bias_sb[name] = []
                for mt in range(D_LOC // P):
                    t = const.tile([P, 1], f32, tag=f"b{name}{mt}",
                                   name=f"b{name}{mt}")
                    nc.sync.dma_start(out=t[:], in_=src_ap[ts(mt, P), :])
                    bias_sb[name].append(t)
            nc.sync.dma_start(out=w_tiles["v"][:, :, :],
                              in_=wvT[:].rearrange("p (k n) -> p k n", k=KT))
            wp_sb = []
            for i in range(D_LOC // P):
                t = const.tile([P, DIM], dt_mm, tag=f"wp{i}", name=f"wp{i}")
                nc.scalar.dma_start(out=t[:], in_=wpT[ts(i, P), :])
                wp_sb.append(t)

            w_sb = {name: [w_tiles[name][:, i, :] for i in range(KT)]
                    for name in ("k", "q", "v")}

            def xs(kt, c):
                return x_sb[:, kt, ts(c, CH)]

            qk_sb = {}
            for name in ("q", "k"):
                qk_sb[name] = [
                    const.tile([P, N_TOK], dt_mm, tag=f"{name}T{mt}",
                               name=f"{name}T{mt}")
                    for mt in range(D_LOC // P)
                ]
            # v packed per key block, ones-augmented: [:, h, 0:64] = v head h,
            # [:, h, 64:128] = 1.0, so each head's PV matmul also produces the
            # softmax denominator (replicated on psum partitions 64:128) in
            # the otherwise-idle half of the PE output -- no den matmuls.
            vpk_sb = [
                const.tile([P, N_HEADS_LOC, P], dt_mm, tag=f"vp{nt}",
                           name=f"vp{nt}")
                for nt in range(MB)
            ]
            for nt in range(MB):
                nc.vector.memset(vpk_sb[nt][:, :, 64:P], 1.0)
            at_sb = {}

            # PE warm-up during the DMA head: HAM needs ~3.4us of sustained
            # activity to unthrottle (1.2 -> 2.4 GHz); without it the first
            # ~9 pass-1 matmuls run at half clock.  Reads the vpk ones
            # (memset, ready ~7us) against the first wk slice (lands ~9.5us)
            # so the array is warm and stays warm until x0 arrives (~15us).
            warm0 = ps_pj.tile([P, CH], f32, tag="pj", name="pj_warm0")
            N_WARM0 = 20
            for i_w in range(N_WARM0):
                nc.tensor.matmul(warm0[0:64, 0:D_LOC],
                                 lhsT=vpk_sb[0][:, 0, 64:P],
                                 rhs=w_sb["k"][0],
                                 start=(i_w == 0), stop=(i_w == N_WARM0 - 1),
                                 skip_group_check=True)
            warm0_sink = work.tile([P, 64], f32, tag="wsink", bufs=2,
                                   name="warm0_sink")
            nc.vector.tensor_copy(warm0_sink[0:1, 0:8], warm0[0:1, 0:8])
            nc.gpsimd.dma_start(out=wsink_dram[0:1, :], in_=warm0_sink[0:1, 0:8])

            # ---- pass 1 (the DMA window): k-mt0 and q-mt0 over all 4
            # chunks, kt-major so emission tracks x arrival; consecutive
            # chunk matmuls share lhsT.  8 psum banks: k on the score pool,
            # q on pj+po.
            s_t = [ps_s.tile([P, 2 * CH], f32, tag="s", name=f"s_p1{i}")
                   for i in range(2)]
            k0_acc = [s_t[0][:, 0:CH], s_t[0][:, CH:2 * CH],
                      s_t[1][:, 0:CH], s_t[1][:, CH:2 * CH]]
            pj_t = [ps_pj.tile([P, CH], f32, tag="pj", name=f"pj_p1{i}")
                    for i in range(2)]
            po_t = [ps_po.tile([P, CH], f32, tag=tg, name=f"po_p1{tg}")
                    for tg in ("poA", "poB")]
            q0_acc = [pj_t[0][:], pj_t[1][:], po_t[0][:], po_t[1][:]]
            for kt in range(KT):
                st, sp = (kt == 0), (kt == KT - 1)
                for c in range(NCH):
                    nc.tensor.matmul(k0_acc[c], lhsT=w_sb["k"][kt][:, 0:P],
                                     rhs=xs(kt, c), start=st, stop=sp)
                for c in range(NCH):
                    nc.tensor.matmul(q0_acc[c], lhsT=w_sb["q"][kt][:, 0:P],
                                     rhs=xs(kt, c), start=st, stop=sp)
            # biases: k-c0 (gates score block 0) and q-c0 first, on separate
            # engines; remaining chunks follow (needed one block later each).
            nc.vector.tensor_scalar_add(qk_sb["k"][0][:, ts(0, CH)],
                                        k0_acc[0], bias_sb["k"][0][:])
            nc.scalar.add(qk_sb["q"][0][:, ts(0, CH)], q0_acc[0],
                          bias_sb["q"][0][:])
            nc.vector.tensor_scalar_add(qk_sb["k"][0][:, ts(1, CH)],
                                        k0_acc[1], bias_sb["k"][0][:])
            nc.scalar.add(qk_sb["k"][0][:, ts(2, CH)], k0_acc[2],
                          bias_sb["k"][0][:])
            nc.vector.tensor_scalar_add(qk_sb["k"][0][:, ts(3, CH)],
                                        k0_acc[3], bias_sb["k"][0][:])
            nc.scalar.add(qk_sb["q"][0][:, ts(1, CH)], q0_acc[1],
                          bias_sb["q"][0][:])
            nc.vector.tensor_scalar_add(qk_sb["q"][0][:, ts(2, CH)],
                                        q0_acc[2], bias_sb["q"][0][:])
            nc.scalar.add(qk_sb["q"][0][:, ts(3, CH)], q0_acc[3],
                          bias_sb["q"][0][:])

            # ---- emission units; generators double as pipeline fillers ----
            def emit_vproj(nt):
                """One v-projection group (one key block), ones kept intact."""
                ps = ps_pj.tile([P, CH], f32, tag="pj", name=f"pj_v{nt}")
                for kt in range(KT):
                    nc.tensor.matmul(
                        ps[:, 0:D_LOC],
                        lhsT=x_sb[:, kt, ts(nt, P)],
                        rhs=w_sb["v"][kt][:],
                        start=(kt == 0), stop=(kt == KT - 1),
                    )
                # single strided copy for all 4 heads (4 separate [128,64]
                # CASTs cost ~350ns each in per-op overhead)
                nc.vector.tensor_copy(
                    vpk_sb[nt][:, :, 0:64],
                    ps[:, 0:D_LOC].rearrange("p (h d) -> p h d", h=N_HEADS_LOC))

            def gen_vproj(nts):
                for nt in nts:
                    emit_vproj(nt)
                    yield

            def gen_proj_pair(name, mt, cpair):
                """mt=1 k/q projection for chunks (2*cpair, 2*cpair+1):
                8 yields, kt-major, consecutive matmuls share lhsT; biases
                on DVE (ScalarE is the stream pacer)."""
                qa = [ps_pj.tile([P, CH], f32, tag="pj",
                                 name=f"pj_{name}{mt}{cpair}{c}")
                      for c in range(2)]
                for kt in range(KT):
                    st, sp = (kt == 0), (kt == KT - 1)
                    for c in range(2):
                        nc.tensor.matmul(
                            qa[c][:], lhsT=w_sb[name][kt][:, ts(mt, P)],
                            rhs=xs(kt, 2 * cpair + c), start=st, stop=sp)
                        yield   # 1 MM per block keeps every block under pace
                for c in range(2):
                    nc.vector.tensor_scalar_add(
                        qk_sb[name][mt][:, ts(2 * cpair + c, CH)],
                        qa[c][:], bias_sb[name][mt][:])

            def gen_outproj(ch, step=1):
                """Output projection for chunk ch; yields every `step` mm."""
                at_tiles = at_sb[ch]
                n = 0
                for mo in range(DIM // P):
                    pp = ps_pj.tile([P, CH], f32, tag="pj", name=f"pj_o{ch}{mo}")
                    for dt_i in range(2):
                        nc.tensor.matmul(
                            pp[:],
                            lhsT=wp_sb[dt_i][:, ts(mo, P)],
                            rhs=at_tiles[dt_i][:],
                            start=(dt_i == 0), stop=(dt_i == 1),
                        )
                        n += 1
                        if n % step == 0:
                            yield
                    os_sb = work.tile([P, CH], dt_mm, tag="os", bufs=8,
                                      name=f"os{ch}{mo}")
                    nc.vector.tensor_copy(os_sb[:], pp[:])
                    nc.sync.dma_start(out=outT[ts(mo, P), ts(ch, CH)],
                                      in_=os_sb[:])

            def run(gen):
                for _ in gen:
                    pass

            # ---- flat software-pipelined stream over all key blocks ----
            # QK+exp lead PV by one block globally, so the in-order PE
            # always has score work queued while ACT runs exp, including
            # across (chunk, head-pair) boundaries.  All h2=0 pairs run
            # first: the mt=1 k/q projections then have positions 1-3 of
            # runway with only the 2-buf pj psum pool, and the outproj for
            # chunk c can start once (c,1) completes (positions 5-7).
            SEQ = [(0, 0), (1, 0), (2, 0), (3, 0),
                   (0, 1), (1, 1), (2, 1), (3, 1)]

            fillers = {
                0: gen_vproj(range(MB)),
                1: gen_proj_pair("k", 1, 0),
                2: gen_proj_pair("k", 1, 1),
                3: gen_proj_pair("q", 1, 0),
                4: gen_proj_pair("q", 1, 1),
                5: None,   # assigned below once at_sb[ch] exists
                6: None,
                7: None,
            }

            blocks = [(i, c, h, mb) for i, (c, h) in enumerate(SEQ)
                      for mb in range(MB)]
            pts = {}
            po_pd = {}
            tail_slots = None
            for g in range(len(blocks) + 1):
                if g == len(blocks):
                    # Pre-emit the first two tail at(3,0)-half matmuls ahead
                    # of the final PV pair: the static scheduler sequences
                    # per-engine in emission order, so anything emitted after
                    # the last PV waits out the whole normalize chain even
                    # without a data dependency.  These keep the PE busy
                    # (HAM warm) while DVE runs the final normalize.
                    st_t = [ps_s.tile([P, 2 * CH], f32, tag="s",
                                      name=f"st{w}") for w in range(2)]
                    tail_slots = [st_t[0][:, 0:CH], st_t[0][:, CH:2 * CH],
                                  st_t[1][:, 0:CH], st_t[1][:, CH:2 * CH]]
                    for j in range(2):
                        nc.tensor.matmul(tail_slots[j],
                                         lhsT=wp_sb[0][:, ts(j, P)],
                                         rhs=at_sb[3][0][:],
                                         start=True, stop=False)
                if g < len(blocks):
                    i, c, h, mb = blocks[g]
                    if mb == 0:
                        if i == 5:
                            fillers[5] = gen_outproj(0)
                        elif i == 6:
                            fillers[6] = gen_outproj(1)
                        elif i == 7:
                            fillers[7] = gen_outproj(2)
                        po_pd[(c, h)] = (
                            ps_po.tile([P, CH], f32, tag="poA", name=f"poA{c}{h}"),
                            ps_po.tile([P, CH], f32, tag="poB", name=f"poB{c}{h}"),
                        )
                    f = fillers.get(i)
                    # outproj fillers wait one block for the preceding
                    # pair's normalize to be emitted.  Positions >=1 pump an
                    # extra yield at mb 3/7/11 so each generator exhausts by
                    # ~mb 12-13: its trailing DVE work (bias adds, last
                    # outproj copy) otherwise lands at the position boundary
                    # ahead of the next pair's poS copies, stalling the
                    # in-order PE behind PV(mb0) (177/393ns ACT gaps/seam).
                    if f is not None and (i < 5 or mb >= 1):
                        next(f, None)
                        if i >= 1 and mb in (3, 7, 11):
                            next(f, None)
                    ps = ps_s.tile([P, 1024], f32, tag="s", name=f"s{c}{h}{mb}")
                    nc.tensor.matmul(
                        ps[:, 0:CH],
                        lhsT=qk_sb["k"][h][0:64, ts(mb, P)],
                        rhs=qk_sb["q"][h][0:64, ts(c, CH)],
                    )
                    nc.tensor.matmul(
                        ps[:, CH:1024],
                        lhsT=qk_sb["k"][h][64:P, ts(mb, P)],
                        rhs=qk_sb["q"][h][64:P, ts(c, CH)],
                    )
                    pt = work.tile([P, 1024], dt_mm, tag="pt", bufs=4,
                                   name=f"pt{c}{h}{mb}")
                    if g == len(blocks) - 1:
                        # split the last exp so the final PV pair (and the
                        # normalize chain behind it) starts half an ACT early
                        nc.scalar.activation(pt[:, 0:CH], ps[:, 0:CH], Exp,
                                             scale=SCALE)
                        nc.scalar.activation(pt[:, CH:1024], ps[:, CH:1024],
                                             Exp, scale=SCALE)
                    else:
                        # NOTE: offloading a fraction of exps to DVE via the
                        # bf16 bit-trick (tensor_scalar into an int16 view,
                        # EXP_A/EXP_B above) is numerically fine (~1% err)
                        # but measured SLOWER (v5 210.9us, v6 215.2us vs
                        # 203.7us): scores(b+2) waits exp(b) through the
                        # 2-deep psum rotation, and on the static schedule a
                        # DVE exp queues behind copies/normalize work --
                        # while any copy moved onto ScalarE head-of-line
                        # blocks later exps in its strict FIFO.  Keeping
                        # ScalarE = exps only is the robust split.
                        nc.scalar.activation(pt[:], ps[:], Exp, scale=SCALE)
                    pts[(c, h, mb)] = pt
                    if g == len(blocks) - 1:
                        last_pt = pt
                    if mb == MB - 1 and f is not None:
                        run(f)   # drain deferred work before leaving position
                if g >= 1:
                    i2, c2, h2, mb2 = blocks[g - 1]
                    poA, poB = po_pd[(c2, h2)]
                    pt = pts.pop((c2, h2, mb2))
                    st = (mb2 == 0)
                    sp = (mb2 == MB - 1)
                    nc.tensor.matmul(
                        poA[:], lhsT=vpk_sb[mb2][:, 2 * h2, :],
                        rhs=pt[:, 0:CH], start=st, stop=sp,
                    )
                    nc.tensor.matmul(
                        poB[:], lhsT=vpk_sb[mb2][:, 2 * h2 + 1, :],
                        rhs=pt[:, CH:1024], start=st, stop=sp,
                    )
                    if sp:
                        # normalize.  HW constraints (micro-tested): two-input
                        # DVE ops need equal input base partitions (out may
                        # shift); reciprocal_approx_fast needs base 0.  So:
                        # stage po to SBUF (also frees the psum slot for the
                        # next pair), cross-copy the den replicas to base 0,
                        # one recip, two aligned muls.  The last pair runs
                        # per-head chains so at[0:64] completes early and the
                        # tail's outproj starts during the second half.
                        del po_pd[(c2, h2)]
                        last = (c2, h2) == SEQ[-1]
                        den = work.tile([64, 2 * CH], f32, tag="den", bufs=2,
                                        name=f"den{c2}{h2}")
                        rec = work.tile([64, 2 * CH], f32, tag="bc", bufs=2,
                                        name=f"rec{c2}{h2}")
                        at = work.tile([P, CH], dt_mm, tag="at", bufs=8,
                                       name=f"at{c2}{h2}")
                        if last:
                            # per-head chains so at[0:64] completes early;
                            # den copies must stay on DVE (ACT reads with a
                            # partition shift fault the exec unit)
                            nc.vector.tensor_copy(den[:, 0:CH], poA[64:P, :])
                            nc.vector.reciprocal_approx_fast(rec[:, 0:CH],
                                                             den[:, 0:CH])
                            nc.vector.tensor_mul(at[0:64, :], poA[0:64, :],
                                                 rec[:, 0:CH])
                            nc.vector.tensor_copy(den[:, CH:2 * CH],
                                                  poB[64:P, :])
                            nc.vector.reciprocal_approx_fast(
                                rec[:, CH:2 * CH], den[:, CH:2 * CH])
                            nc.vector.tensor_mul(at[64:P, :], poB[0:64, :],
                                                 rec[:, CH:2 * CH])
                        else:
                            poS = work.tile([P, 2 * CH], f32, tag="poS",
                                            bufs=2, name=f"poS{c2}{h2}")
                            nc.vector.tensor_copy(poS[:, 0:CH], poA[:])
                            nc.vector.tensor_copy(poS[:, CH:2 * CH], poB[:])
                            nc.vector.tensor_copy(den[:], poS[64:P, :])
                            nc.vector.reciprocal_approx_fast(rec[:], den[:])
                            nc.vector.tensor_mul(at[0:64, :],
                                                 poS[0:64, 0:CH],
                                                 rec[:, 0:CH])
                            nc.vector.tensor_mul(at[64:P, :],
                                                 poS[0:64, CH:2 * CH],
                                                 rec[:, CH:2 * CH])
                        at_sb.setdefault(c2, []).append(at)
            # ---- tail: chunk-3 outproj, K=256 accumulated per mo tile.
            # mo 0-3 on the freed score banks (first two K0s pre-emitted
            # above), mo 4-5 on the pj ring -- six at(3,0)-half matmuls
            # execute during the final normalize so HAM stays at 8/8 --
            # then the at(3,1)-half matmuls stream back-to-back.  Copies
            # alternate Vector/Scalar (both idle now); DMA issues
            # round-robin 3 engines.
            at0, at1 = at_sb[3]
            dma_engs = (nc.sync, nc.scalar, nc.gpsimd)
            for j in (2, 3):
                nc.tensor.matmul(tail_slots[j], lhsT=wp_sb[0][:, ts(j, P)],
                                 rhs=at0[:], start=True, stop=False)
            # Warm matmuls: the PE is otherwise sparse in the HAM MID window
            # spanning the final normalize, which re-throttles the clock to
            # 1.2GHz and slows the whole tail (~584ns/MM measured).  These
            # read the LAST block's exp output so the scheduler cannot hoist
            # them, and sink one value to DRAM so nothing is eliminated.
            # Emitted BEFORE the pjt allocations: a pj tile allocated first
            # would make the warm group wait on its own downstream readers.
            N_WARM = 6
            wpp = ps_pj.tile([P, CH], f32, tag="pj", name="pj_warm")
            for i_w in range(N_WARM):
                nc.tensor.matmul(wpp[:], lhsT=wp_sb[0][:, 0:P],
                                 rhs=last_pt[:, 0:CH],
                                 start=(i_w == 0), stop=(i_w == N_WARM - 1),
                                 skip_group_check=True)
            warm_sink = work.tile([P, 64], f32, tag="wsink", bufs=2,
                                  name="warm_sink")
            nc.vector.tensor_copy(warm_sink[0:1, 0:8], wpp[0:1, 0:8])
            nc.sync.dma_start(out=wsink_dram[1:2, :], in_=warm_sink[0:1, 0:8])
            pjt = [ps_pj.tile([P, CH], f32, tag="pj", name=f"pj_t{j}")
                   for j in range(2)]
            for j in range(2):
                nc.tensor.matmul(pjt[j][:], lhsT=wp_sb[0][:, ts(4 + j, P)],
                                 rhs=at0[:], start=True, stop=False)
            tail_slots += [pjt[0][:], pjt[1][:]]

            def tail_fin(mo, slot):
                nc.tensor.matmul(slot, lhsT=wp_sb[1][:, ts(mo, P)],
                                 rhs=at1[:], start=False, stop=True)
                os_sb = work.tile([P, CH], dt_mm, tag="os", bufs=8,
                                  name=f"os3{mo}")
                if mo % 2 == 0:
                    nc.vector.tensor_copy(os_sb[:], slot)
                else:
                    nc.scalar.copy(os_sb[:], slot)
                dma_engs[mo % 3].dma_start(out=outT[ts(mo, P), ts(3, CH)],
                                           in_=os_sb[:])

            for mo in range(6):
                tail_fin(mo, tail_slots[mo])
            st_w = ps_s.tile([P, 2 * CH], f32, tag="s", name="st_w1")
            for j, mo in enumerate((6, 7)):
                nc.tensor.matmul(st_w[:, ts(j, CH)],
                                 lhsT=wp_sb[0][:, ts(mo, P)],
                                 rhs=at0[:], start=True, stop=False)
            for j, mo in enumerate((6, 7)):
                tail_fin(mo, st_w[:, ts(j, CH)])

    nc.compile()
    return nc


def _get_nc():
    if "nc" not in _NC_CACHE:
        _NC_CACHE["nc"] = build_nc(DT_MM_NAME)
    return _NC_CACHE["nc"]


def make_in_maps(x, Wq, bq, Wk, bk, Wv, bv, Wp, bp, dt_mm_name="bfloat16"):
    """Shard full inputs into 8 per-core input maps."""
    f = np.float32
    if dt_mm_name == "bfloat16":
        import ml_dtypes
        mmt = ml_dtypes.bfloat16
    else:
        mmt = np.float32
    x = np.asarray(x, f)
    xT = [np.ascontiguousarray(x[b].T).astype(mmt) for b in range(x.shape[0])]
    WqT = np.asarray(Wq, f).T
    WkT = np.asarray(Wk, f).T
    WvT = np.asarray(Wv, f).T
    WpT = np.asarray(Wp, f).T
    def pretile(w):
        # [1024, 256] -> [128, 8*256]: partition p holds all 8 k-tiles
        # contiguously so DMA descriptors are 4KB DRAM runs
        return np.ascontiguousarray(
            w.reshape(KT, P, D_LOC).transpose(1, 0, 2).reshape(P, KT * D_LOC)
        ).astype(mmt)

    in_maps = []
    for c in range(N_CORES):
        b, r = divmod(c, 4)
        sl = slice(D_LOC * r, D_LOC * (r + 1))
        in_maps.append({
            "xT": xT[b],
            "wqT": pretile(WqT[:, sl]),
            "wkT": pretile(WkT[:, sl]),
            "wvT": pretile(WvT[:, sl]),
            "wpT": np.ascontiguousarray(WpT[sl, :]).astype(mmt),
            "bq": np.asarray(bq, f)[sl].reshape(D_LOC, 1).copy(),
            "bk": np.asarray(bk, f)[sl].reshape(D_LOC, 1).copy(),
        })
    return in_maps


def assemble_output(results, Wv, bv, Wp, bp):
    """Sum TP partials, transpose back, add folded biases."""
    f = np.float32
    bp_eff = np.asarray(bv, f) @ np.asarray(Wp, f).T + np.asarray(bp, f)
    out = np.empty((2, N_TOK, DIM), f)
    for b in range(2):
        acc = results[4 * b]["outT"].astype(f)
        for r in range(1, 4):
            acc = acc + results[4 * b + r]["outT"].astype(f)
        out[b] = acc.T + bp_eff
    return out


DT_MM_NAME = "bfloat16"

# Schraudolph-in-bf16 exp constants for the DVE-offloaded blocks:
# bf16 bits of 2^y ~= 128*y + 128*(127 - 0.043); with y = s*SCALE*log2(e)
# one DVE tensor_scalar (mult, add) writing int16 into the pt tile computes
# exp(s*SCALE) to ~2% RMS (common mode removed by the softmax normalize).
EXP_A = 128.0 * 1.4426950408889634 * SCALE
EXP_B = 16250.75


def kernel(x, Wq, bq, Wk, bk, Wv, bv, Wp, bp):
    from concourse.bass_utils import run_bass_kernel_spmd
    nc = _get_nc()
    in_maps = make_in_maps(x, Wq, bq, Wk, bk, Wv, bv, Wp, bp, DT_MM_NAME)
    res = run_bass_kernel_spmd(nc, in_maps, list(range(N_CORES)))
    return assemble_output(res.results, Wv, bv, Wp, bp)


# revision 46
# speedup vs baseline: 1.0182x; 1.0182x over previous
"""Fused multi-head attention on 8 TRN2 NeuronCores.

Problem: x[2,2048,1024] -> q,k,v = x@W.T+b (16 heads x 64), softmax(q k^T/8) v,
then out @ Wp.T + bp.

Sharding: data-parallel over batch (2) x tensor-parallel over heads (4 ranks x
4 heads = 256 dims, Megatron-style).  Core c handles batch c//4, head-rank c%4.
The proj partial sums are reduced on the host (numpy), and the v-bias and
proj-bias are folded into one host-side vector bp_eff = bv @ Wp.T + bp.

Per-core layouts (host pre-transposes/pre-tiles, all DMA rows are >=512B
contiguous DRAM runs):
  xT  [1024, 2048]  x[b].T
  wqT/wkT/wvT [128, 8*256]  W.T slice pre-tiled so partition p holds all 8
                            contraction tiles contiguously
  wpT [256, 1024]           Wp.T rows for this rank's 256 dims
  bq/bk [256, 1]
  outT [1024, 2048] bf16 partial (x[b] @ ..).T, missing bv/bp contributions

Schedule (all matmul operands bfloat16, fp32 PSUM accumulate):
  head:   x arrives as 4x 1MB DMAs (2 k-tiles each).  The k/q mt=0
          projections accumulate kt-major across all 8 PSUM banks, paced
          by x arrival, so the DMA window is PE-busy.  Biases are applied
          psum->SBUF with ScalarE activation(Identity, bias) and DVE
          tensor_scalar_add in parallel, k-chunk0/q-chunk0 first so the
          attention stream starts immediately after the last x tile.
  stream: ONE flat software-pipelined stream over all 128 (n-chunk,
          head-pair, key-block) blocks; per block:
            sT[m, n] = kT.T @ qT   two heads row-packed (concurrent K=64
                                   matmuls in distinct PE row groups)
            p = exp(sT / 8)        ACT, one [128,1024] instr, both heads
            po_h[:, n] += vpk[mb,h].T @ p_h  per head: psum partitions
                                   0:64 = P@V, 64:128 = softmax denominator
                                   (ones-columns of vpk) -- no den matmuls.
          PV trails its block's exp by one position globally.  The stream
          is ACT-paced (~1.16us/block); fillers pumped per block absorb
          the PE slack: v-projection (position 0, JIT per key block),
          k/q mt=1 projections with chunk-interleaved lhsT reuse
          (positions 1-2), output projection chunks 0-2 (positions 5-7).
  attnT = po[0:64] * reciprocal_approx_fast(po[64:128]) per head
  tail:   chunk-3 outproj runs as 8 K=256 psum-accumulated groups (no
          SBUF staging / no DVE adds); psum->SBUF copies alternate
          Vector/Scalar, output DMAs round-robin 3 engines; the K-half
          reading at(3,0) starts during the final normalize.
"""

import numpy as np

DIM = 1024
N_TOK = 2048
N_HEADS_LOC = 4       # heads per core
D_LOC = 256           # local q/k/v dims per core
SCALE = 64 ** -0.5
P = 128
CH = 512              # n-chunk (moving free dim)
NCH = N_TOK // CH     # 4
KT = DIM // P         # 8 contraction tiles for qkv/proj
MB = N_TOK // P       # 16 key blocks
N_CORES = 8

_NC_CACHE = {}


def build_nc(dt_mm_name="bfloat16"):
    import concourse.mybir as mybir
    import concourse.tile as tile
    from concourse import bacc
    from concourse.bass import ts

    f32 = mybir.dt.float32
    dt_mm = getattr(mybir.dt, dt_mm_name)
    Exp = mybir.ActivationFunctionType.Exp

    nc = bacc.Bacc("TRN2", target_bir_lowering=False, debug=False,
                   num_devices=N_CORES)
    xT = nc.dram_tensor("xT", [DIM, N_TOK], dt_mm, kind="ExternalInput").ap()
    wqT = nc.dram_tensor("wqT", [P, KT * D_LOC], dt_mm, kind="ExternalInput").ap()
    wkT = nc.dram_tensor("wkT", [P, KT * D_LOC], dt_mm, kind="ExternalInput").ap()
    wvT = nc.dram_tensor("wvT", [P, KT * D_LOC], dt_mm, kind="ExternalInput").ap()
    wpT = nc.dram_tensor("wpT", [D_LOC, DIM], dt_mm, kind="ExternalInput").ap()
    bq = nc.dram_tensor("bq", [D_LOC, 1], f32, kind="ExternalInput").ap()
    bk = nc.dram_tensor("bk", [D_LOC, 1], f32, kind="ExternalInput").ap()
    outT = nc.dram_tensor("outT", [DIM, N_TOK], dt_mm, kind="ExternalOutput").ap()
    wsink_dram = nc.dram_tensor("wsink", [2, 8], f32, kind="ExternalOutput").ap()

    with tile.TileContext(nc) as tc:
        with (
            tc.tile_pool(name="const", bufs=1) as const,
            tc.tile_pool(name="work", bufs=2) as work,
            # scores: 2x [128,1024] = 4 banks; also pass-1 k-mt0 accums
            tc.tile_pool(name="ps_s", bufs=2, space="PSUM") as ps_s,
            # filler ring (projections, vproj, outproj): 2x [128,512]
            tc.tile_pool(name="ps_pj", bufs=2, space="PSUM") as ps_pj,
            # PV+den accumulators, one per head of the active pair: 2 banks
            tc.tile_pool(name="ps_po", bufs=1, space="PSUM") as ps_po,
        ):
            # ---- persistent SBUF state ----
            w_tiles = {}
            for name in ("k", "q", "v"):
                w_tiles[name] = const.tile([P, KT, D_LOC], dt_mm, tag=f"w{name}",
                                           name=f"w{name}")
            x_sb = const.tile([P, KT, N_TOK], dt_mm, tag="x", name="x")

            # DMA issue order: each engine's dma_starts serialize (~0.65us
            # each) and the queues round-robin in-flight transfers at packet
            # granularity, so total wire time (~6.3MB at ~300GB/s = 21us)
            # bounds the head.  x goes as 4x 1MB transfers (128-partition,
            # >=1MB is the max-bandwidth shape); wv is issued first on the
            # vector queue so the position-0 v-projection isn't gated at
            # stream start; wp (needed at position 3) queues last.
            def w_slice(dst, src_ap, k0, k1):
                nc.sync.dma_start(
                    out=dst[:, k0:k1, :],
                    in_=src_ap[:, k0 * D_LOC:k1 * D_LOC].rearrange(
                        "p (k n) -> p k n", k=k1 - k0))

            # x: kt0 and kt1 as 512KB transfers (kt0 gates the first pass-1
            # matmuls), the rest as 1MB; wv rides the END of the (short)
            # sync queue so it lands ~23us, before the position-0
            # v-projection needs it; queue loads balance to ~2-2.5MB each.
            def x_slice(k0, k1):
                return xT[k0 * P:k1 * P, :].rearrange(
                    "(k p) n -> p k n", k=k1 - k0)
            nc.gpsimd.dma_start(out=x_sb[:, 0:1, :], in_=x_slice(0, 1))
            nc.scalar.dma_start(out=x_sb[:, 2:4, :], in_=x_slice(2, 4))
            nc.gpsimd.dma_start(out=x_sb[:, 1:2, :], in_=x_slice(1, 2))
            w_slice(w_tiles["k"], wkT, 0, 2)
            w_slice(w_tiles["q"], wqT, 0, 2)
            nc.gpsimd.dma_start(out=x_sb[:, 4:6, :], in_=x_slice(4, 6))
            nc.scalar.dma_start(out=x_sb[:, 6:8, :], in_=x_slice(6, 8))
            w_slice(w_tiles["k"], wkT, 2, 4)
            w_slice(w_tiles["q"], wqT, 2, 4)
            w_slice(w_tiles["k"], wkT, 4, 6)
            w_slice(w_tiles["q"], wqT, 4, 6)
            w_slice(w_tiles["k"], wkT, 6, 8)
            w_slice(w_tiles["q"], wqT, 6, 8)
            bias_sb = {}
            for name, src_ap in (("q", bq), ("k", bk)):
                bias_sb[name] = []
                for mt in range(D_LOC // P):
                    t = const.tile([P, 1], f32, tag=f"b{name}{mt}",
                                   name=f"b{name}{mt}")
                    nc.sync.dma_start(out=t[:], in_=src_ap[ts(mt, P), :])
                    bias_sb[name].append(t)
            nc.sync.dma_start(out=w_tiles["v"][:, :, :],
                              in_=wvT[:].rearrange("p (k n) -> p k n", k=KT))
            wp_sb = []
            for i in range(D_LOC // P):
                t = const.tile([P, DIM], dt_mm, tag=f"wp{i}", name=f"wp{i}")
                nc.scalar.dma_start(out=t[:], in_=wpT[ts(i, P), :])
                wp_sb.append(t)

            w_sb = {name: [w_tiles[name][:, i, :] for i in range(KT)]
                    for name in ("k", "q", "v")}

            def xs(kt, c):
                return x_sb[:, kt, ts(c, CH)]

            qk_sb = {}
            for name in ("q", "k"):
                qk_sb[name] = [
                    const.tile([P, N_TOK], dt_mm, tag=f"{name}T{mt}",
                               name=f"{name}T{mt}")
                    for mt in range(D_LOC // P)
                ]
            # v packed per key block, ones-augmented: [:, h, 0:64] = v head h,
            # [:, h, 64:128] = 1.0, so each head's PV matmul also produces the
            # softmax denominator (replicated on psum partitions 64:128) in
            # the otherwise-idle half of the PE output -- no den matmuls.
            vpk_sb = [
                const.tile([P, N_HEADS_LOC, P], dt_mm, tag=f"vp{nt}",
                           name=f"vp{nt}")
                for nt in range(MB)
            ]
            for nt in range(MB):
                nc.vector.memset(vpk_sb[nt][:, :, 64:P], 1.0)
            at_sb = {}

            # PE warm-up during the DMA head: HAM needs ~3.4us of sustained
            # activity to unthrottle (1.2 -> 2.4 GHz); without it the first
            # ~9 pass-1 matmuls run at half clock.  Reads the vpk ones
            # (memset, ready ~7us) against the first wk slice (lands ~9.5us)
            # so the array is warm and stays warm until x0 arrives (~15us).
            warm0 = ps_pj.tile([P, CH], f32, tag="pj", name="pj_warm0")
            N_WARM0 = 20
            for i_w in range(N_WARM0):
                nc.tensor.matmul(warm0[0:64, 0:D_LOC],
                                 lhsT=vpk_sb[0][:, 0, 64:P],
                                 rhs=w_sb["k"][0],
                                 start=(i_w == 0), stop=(i_w == N_WARM0 - 1),
                                 skip_group_check=True)
            warm0_sink = work.tile([P, 64], f32, tag="wsink", bufs=2,
                                   name="warm0_sink")
            nc.vector.tensor_copy(warm0_sink[0:1, 0:8], warm0[0:1, 0:8])
            nc.gpsimd.dma_start(out=wsink_dram[0:1, :], in_=warm0_sink[0:1, 0:8])

            # ---- pass 1 (the DMA window): k-mt0 and q-mt0 over all 4
            # chunks, kt-major so emission tracks x arrival; consecutive
            # chunk matmuls share lhsT.  8 psum banks: k on the score pool,
            # q on pj+po.
            s_t = [ps_s.tile([P, 2 * CH], f32, tag="s", name=f"s_p1{i}")
                   for i in range(2)]
            k0_acc = [s_t[0][:, 0:CH], s_t[0][:, CH:2 * CH],
                      s_t[1][:, 0:CH], s_t[1][:, CH:2 * CH]]
            pj_t = [ps_pj.tile([P, CH], f32, tag="pj", name=f"pj_p1{i}")
                    for i in range(2)]
            po_t = [ps_po.tile([P, CH], f32, tag=tg, name=f"po_p1{tg}")
                    for tg in ("poA", "poB")]
            q0_acc = [pj_t[0][:], pj_t[1][:], po_t[0][:], po_t[1][:]]
            for kt in range(KT):
                st, sp = (kt == 0), (kt == KT - 1)
                for c in range(NCH):
                    nc.tensor.matmul(k0_acc[c], lhsT=w_sb["k"][kt][:, 0:P],
                                     rhs=xs(kt, c), start=st, stop=sp)
                for c in range(NCH):
                    nc.tensor.matmul(q0_acc[c], lhsT=w_sb["q"][kt][:, 0:P],
                                     rhs=xs(kt, c), start=st, stop=sp)
            # biases: k-c0 (gates score block 0) and q-c0 first, on separate
            # engines; remaining chunks follow (needed one block later each).
            nc.vector.tensor_scalar_add(qk_sb["k"][0][:, ts(0, CH)],
                                        k0_acc[0], bias_sb["k"][0][:])
            nc.scalar.add(qk_sb["q"][0][:, ts(0, CH)], q0_acc[0],
                          bias_sb["q"][0][:])
            nc.vector.tensor_scalar_add(qk_sb["k"][0][:, ts(1, CH)],
                                        k0_acc[1], bias_sb["k"][0][:])
            nc.scalar.add(qk_sb["k"][0][:, ts(2, CH)], k0_acc[2],
                          bias_sb["k"][0][:])
            nc.vector.tensor_scalar_add(qk_sb["k"][0][:, ts(3, CH)],
                                        k0_acc[3], bias_sb["k"][0][:])
            nc.scalar.add(qk_sb["q"][0][:, ts(1, CH)], q0_acc[1],
                          bias_sb["q"][0][:])
            nc.vector.tensor_scalar_add(qk_sb["q"][0][:, ts(2, CH)],
                                        q0_acc[2], bias_sb["q"][0][:])
            nc.scalar.add(qk_sb["q"][0][:, ts(3, CH)], q0_acc[3],
                          bias_sb["q"][0][:])

            # ---- emission units; generators double as pipeline fillers ----
            def emit_vproj(nt):
                """One v-projection group (one key block), ones kept intact."""
                ps = ps_pj.tile([P, CH], f32, tag="pj", name=f"pj_v{nt}")
                for kt in range(KT):
                    nc.tensor.matmul(
                        ps[:, 0:D_LOC],
                        lhsT=x_sb[:, kt, ts(nt, P)],
                        rhs=w_sb["v"][kt][:],
                        start=(kt == 0), stop=(kt == KT - 1),
                    )
                # single strided copy for all 4 heads (4 separate [128,64]
                # CASTs cost ~350ns each in per-op overhead)
                nc.vector.tensor_copy(
                    vpk_sb[nt][:, :, 0:64],
                    ps[:, 0:D_LOC].rearrange("p (h d) -> p h d", h=N_HEADS_LOC))

            def gen_vproj(nts):
                for nt in nts:
                    emit_vproj(nt)
                    yield

            def gen_proj_pair(name, mt, cpair):
                """mt=1 k/q projection for chunks (2*cpair, 2*cpair+1):
                8 yields, kt-major, consecutive matmuls share lhsT; biases
                on DVE (ScalarE is the stream pacer)."""
                qa = [ps_pj.tile([P, CH], f32, tag="pj",
                                 name=f"pj_{name}{mt}{cpair}{c}")
                      for c in range(2)]
                for kt in range(KT):
                    st, sp = (kt == 0), (kt == KT - 1)
                    for c in range(2):
                        nc.tensor.matmul(
                            qa[c][:], lhsT=w_sb[name][kt][:, ts(mt, P)],
                            rhs=xs(kt, 2 * cpair + c), start=st, stop=sp)
                        yield   # 1 MM per block keeps every block under pace
                for c in range(2):
                    nc.vector.tensor_scalar_add(
                        qk_sb[name][mt][:, ts(2 * cpair + c, CH)],
                        qa[c][:], bias_sb[name][mt][:])

            def gen_outproj(ch, step=1):
                """Output projection for chunk ch; yields every `step` mm."""
                at_tiles = at_sb[ch]
                n = 0
                for mo in range(DIM // P):
                    pp = ps_pj.tile([P, CH], f32, tag="pj", name=f"pj_o{ch}{mo}")
                    for dt_i in range(2):
                        nc.tensor.matmul(
                            pp[:],
                            lhsT=wp_sb[dt_i][:, ts(mo, P)],
                            rhs=at_tiles[dt_i][:],
                            start=(dt_i == 0), stop=(dt_i == 1),
                        )
                        n += 1
                        if n % step == 0:
                            yield
                    os_sb = work.tile([P, CH], dt_mm, tag="os", bufs=8,
                                      name=f"os{ch}{mo}")
                    nc.vector.tensor_copy(os_sb[:], pp[:])
                    nc.sync.dma_start(out=outT[ts(mo, P), ts(ch, CH)],
                                      in_=os_sb[:])

            def run(gen):
                for _ in gen:
                    pass

            # ---- flat software-pipelined stream over all key blocks ----
            # QK+exp lead PV by one block globally, so the in-order PE
            # always has score work queued while ACT runs exp, including
            # across (chunk, head-pair) boundaries.  All h2=0 pairs run
            # first: the mt=1 k/q projections then have positions 1-3 of
            # runway with only the 2-buf pj psum pool, and the outproj for
            # chunk c can start once (c,1) completes (positions 5-7).
            SEQ = [(0, 0), (1, 0), (2, 0), (3, 0),
                   (0, 1), (1, 1), (2, 1), (3, 1)]

            fillers = {
                0: gen_vproj(range(MB)),
                1: gen_proj_pair("k", 1, 0),
                2: gen_proj_pair("k", 1, 1),
                3: gen_proj_pair("q", 1, 0),
                4: gen_proj_pair("q", 1, 1),
                5: None,   # assigned below once at_sb[ch] exists
                6: None,
                7: None,
            }

            blocks = [(i, c, h, mb) for i, (c, h) in enumerate(SEQ)
                      for mb in range(MB)]
            pts = {}
            po_pd = {}
            tail_slots = None
            for g in range(len(blocks) + 1):
                if g == len(blocks):
                    # Pre-emit the first two tail at(3,0)-half matmuls ahead
                    # of the final PV pair: the static scheduler sequences
                    # per-engine in emission order, so anything emitted after
                    # the last PV waits out the whole normalize chain even
                    # without a data dependency.  These keep the PE busy
                    # (HAM warm) while DVE runs the final normalize.
                    st_t = [ps_s.tile([P, 2 * CH], f32, tag="s",
                                      name=f"st{w}") for w in range(2)]
                    tail_slots = [st_t[0][:, 0:CH], st_t[0][:, CH:2 * CH],
                                  st_t[1][:, 0:CH], st_t[1][:, CH:2 * CH]]
                    for j in range(2):
                        nc.tensor.matmul(tail_slots[j],
                                         lhsT=wp_sb[0][:, ts(j, P)],
                                         rhs=at_sb[3][0][:],
                                         start=True, stop=False)
                if g < len(blocks):
                    i, c, h, mb = blocks[g]
                    if mb == 0:
                        if i == 5:
                            fillers[5] = gen_outproj(0)
                        elif i == 6:
                            fillers[6] = gen_outproj(1)
                        elif i == 7:
                            fillers[7] = gen_outproj(2)
                        po_pd[(c, h)] = (
                            ps_po.tile([P, CH], f32, tag="poA", name=f"poA{c}{h}"),
                            ps_po.tile([P, CH], f32, tag="poB", name=f"poB{c}{h}"),
                        )
                    f = fillers.get(i)
                    # outproj fillers wait one block for the preceding
                    # pair's normalize to be emitted.  (Double-pumping at mb
                    # 3/7/11 to drain trailing DVE adds before the position
                    # boundary measured ~4us SLOWER -- the compressed filler
                    # blocks disrupt the static schedule more than the
                    # ~1.5us of boundary ACT stalls cost.  Keep 1/block.)
                    if f is not None and (i < 5 or mb >= 1):
                        next(f, None)
                    ps = ps_s.tile([P, 1024], f32, tag="s", name=f"s{c}{h}{mb}")
                    nc.tensor.matmul(
                        ps[:, 0:CH],
                        lhsT=qk_sb["k"][h][0:64, ts(mb, P)],
                        rhs=qk_sb["q"][h][0:64, ts(c, CH)],
                    )
                    nc.tensor.matmul(
                        ps[:, CH:1024],
                        lhsT=qk_sb["k"][h][64:P, ts(mb, P)],
                        rhs=qk_sb["q"][h][64:P, ts(c, CH)],
                    )
                    pt = work.tile([P, 1024], dt_mm, tag="pt", bufs=4,
                                   name=f"pt{c}{h}{mb}")
                    if g == len(blocks) - 1:
                        # split the last exp so the final PV pair (and the
                        # normalize chain behind it) starts half an ACT early
                        nc.scalar.activation(pt[:, 0:CH], ps[:, 0:CH], Exp,
                                             scale=SCALE)
                        nc.scalar.activation(pt[:, CH:1024], ps[:, CH:1024],
                                             Exp, scale=SCALE)
                    else:
                        # NOTE: offloading a fraction of exps to DVE via the
                        # bf16 bit-trick (tensor_scalar into an int16 view,
                        # EXP_A/EXP_B above) is numerically fine (~1% err)
                        # but measured SLOWER (v5 210.9us, v6 215.2us vs
                        # 203.7us): scores(b+2) waits exp(b) through the
                        # 2-deep psum rotation, and on the static schedule a
                        # DVE exp queues behind copies/normalize work --
                        # while any copy moved onto ScalarE head-of-line
                        # blocks later exps in its strict FIFO.  Keeping
                        # ScalarE = exps only is the robust split.
                        nc.scalar.activation(pt[:], ps[:], Exp, scale=SCALE)
                    pts[(c, h, mb)] = pt
                    if g == len(blocks) - 1:
                        last_pt = pt
                    if mb == MB - 1 and f is not None:
                        run(f)   # drain deferred work before leaving position
                if g >= 1:
                    i2, c2, h2, mb2 = blocks[g - 1]
                    poA, poB = po_pd[(c2, h2)]
                    pt = pts.pop((c2, h2, mb2))
                    st = (mb2 == 0)
                    sp = (mb2 == MB - 1)
                    nc.tensor.matmul(
                        poA[:], lhsT=vpk_sb[mb2][:, 2 * h2, :],
                        rhs=pt[:, 0:CH], start=st, stop=sp,
                    )
                    nc.tensor.matmul(
                        poB[:], lhsT=vpk_sb[mb2][:, 2 * h2 + 1, :],
                        rhs=pt[:, CH:1024], start=st, stop=sp,
                    )
                    if sp:
                        # normalize.  HW constraints (micro-tested): two-input
                        # DVE ops need equal input base partitions (out may
                        # shift); reciprocal_approx_fast needs base 0.  So:
                        # stage po to SBUF (also frees the psum slot for the
                        # next pair), cross-copy the den replicas to base 0,
                        # one recip, two aligned muls.  The last pair runs
                        # per-head chains so at[0:64] completes early and the
                        # tail's outproj starts during the second half.
                        del po_pd[(c2, h2)]
                        last = (c2, h2) == SEQ[-1]
                        den = work.tile([64, 2 * CH], f32, tag="den", bufs=2,
                                        name=f"den{c2}{h2}")
                        rec = work.tile([64, 2 * CH], f32, tag="bc", bufs=2,
                                        name=f"rec{c2}{h2}")
                        at = work.tile([P, CH], dt_mm, tag="at", bufs=8,
                                       name=f"at{c2}{h2}")
                        if last:
                            # per-head chains so at[0:64] completes early;
                            # den copies must stay on DVE (ACT reads with a
                            # partition shift fault the exec unit)
                            nc.vector.tensor_copy(den[:, 0:CH], poA[64:P, :])
                            nc.vector.reciprocal_approx_fast(rec[:, 0:CH],
                                                             den[:, 0:CH])
                            nc.vector.tensor_mul(at[0:64, :], poA[0:64, :],
                                                 rec[:, 0:CH])
                            nc.vector.tensor_copy(den[:, CH:2 * CH],
                                                  poB[64:P, :])
                            nc.vector.reciprocal_approx_fast(
                                rec[:, CH:2 * CH], den[:, CH:2 * CH])
                            nc.vector.tensor_mul(at[64:P, :], poB[0:64, :],
                                                 rec[:, CH:2 * CH])
                        else:
                            poS = work.tile([P, 2 * CH], f32, tag="poS",
                                            bufs=2, name=f"poS{c2}{h2}")
                            nc.vector.tensor_copy(poS[:, 0:CH], poA[:])
                            nc.vector.tensor_copy(poS[:, CH:2 * CH], poB[:])
                            nc.vector.tensor_copy(den[:], poS[64:P, :])
                            nc.vector.reciprocal_approx_fast(rec[:], den[:])
                            nc.vector.tensor_mul(at[0:64, :],
                                                 poS[0:64, 0:CH],
                                                 rec[:, 0:CH])
                            nc.vector.tensor_mul(at[64:P, :],
                                                 poS[0:64, CH:2 * CH],
                                                 rec[:, CH:2 * CH])
                        at_sb.setdefault(c2, []).append(at)
            # ---- tail: chunk-3 outproj, K=256 accumulated per mo tile.
            # mo 0-3 on the freed score banks (first two K0s pre-emitted
            # above), mo 4-5 on the pj ring -- six at(3,0)-half matmuls
            # execute during the final normalize so HAM stays at 8/8 --
            # then the at(3,1)-half matmuls stream back-to-back.  Copies
            # alternate Vector/Scalar (both idle now); DMA issues
            # round-robin 3 engines.
            at0, at1 = at_sb[3]
            dma_engs = (nc.sync, nc.scalar, nc.gpsimd)
            for j in (2, 3):
                nc.tensor.matmul(tail_slots[j], lhsT=wp_sb[0][:, ts(j, P)],
                                 rhs=at0[:], start=True, stop=False)
            # Warm matmuls: the PE is otherwise sparse in the HAM MID window
            # spanning the final normalize, which re-throttles the clock to
            # 1.2GHz and slows the whole tail (~584ns/MM measured).  These
            # read the LAST block's exp output so the scheduler cannot hoist
            # them, and sink one value to DRAM so nothing is eliminated.
            # Emitted BEFORE the pjt allocations: a pj tile allocated first
            # would make the warm group wait on its own downstream readers.
            N_WARM = 6
            wpp = ps_pj.tile([P, CH], f32, tag="pj", name="pj_warm")
            for i_w in range(N_WARM):
                nc.tensor.matmul(wpp[:], lhsT=wp_sb[0][:, 0:P],
                                 rhs=last_pt[:, 0:CH],
                                 start=(i_w == 0), stop=(i_w == N_WARM - 1),
                                 skip_group_check=True)
            warm_sink = work.tile([P, 64], f32, tag="wsink", bufs=2,
                                  name="warm_sink")
            nc.vector.tensor_copy(warm_sink[0:1, 0:8], wpp[0:1, 0:8])
            nc.sync.dma_start(out=wsink_dram[1:2, :], in_=warm_sink[0:1, 0:8])
            pjt = [ps_pj.tile([P, CH], f32, tag="pj", name=f"pj_t{j}")
                   for j in range(2)]
            for j in range(2):
                nc.tensor.matmul(pjt[j][:], lhsT=wp_sb[0][:, ts(4 + j, P)],
                                 rhs=at0[:], start=True, stop=False)
            tail_slots += [pjt[0][:], pjt[1][:]]

            def tail_fin(mo, slot):
                nc.tensor.matmul(slot, lhsT=wp_sb[1][:, ts(mo, P)],
                                 rhs=at1[:], start=False, stop=True)
                os_sb = work.tile([P, CH], dt_mm, tag="os", bufs=8,
                                  name=f"os3{mo}")
                if mo % 2 == 0:
                    nc.vector.tensor_copy(os_sb[:], slot)
                else:
                    nc.scalar.copy(os_sb[:], slot)
                dma_engs[mo % 3].dma_start(out=outT[ts(mo, P), ts(3, CH)],
                                           in_=os_sb[:])

            for mo in range(6):
                tail_fin(mo, tail_slots[mo])
            st_w = ps_s.tile([P, 2 * CH], f32, tag="s", name="st_w1")
            for j, mo in enumerate((6, 7)):
                nc.tensor.matmul(st_w[:, ts(j, CH)],
                                 lhsT=wp_sb[0][:, ts(mo, P)],
                                 rhs=at0[:], start=True, stop=False)
            for j, mo in enumerate((6, 7)):
                tail_fin(mo, st_w[:, ts(j, CH)])

    nc.compile()
    return nc


def _get_nc():
    if "nc" not in _NC_CACHE:
        _NC_CACHE["nc"] = build_nc(DT_MM_NAME)
    return _NC_CACHE["nc"]


def make_in_maps(x, Wq, bq, Wk, bk, Wv, bv, Wp, bp, dt_mm_name="bfloat16"):
    """Shard full inputs into 8 per-core input maps."""
    f = np.float32
    if dt_mm_name == "bfloat16":
        import ml_dtypes
        mmt = ml_dtypes.bfloat16
    else:
        mmt = np.float32
    x = np.asarray(x, f)
    xT = [np.ascontiguousarray(x[b].T).astype(mmt) for b in range(x.shape[0])]
    WqT = np.asarray(Wq, f).T
    WkT = np.asarray(Wk, f).T
    WvT = np.asarray(Wv, f).T
    WpT = np.asarray(Wp, f).T
    def pretile(w):
        # [1024, 256] -> [128, 8*256]: partition p holds all 8 k-tiles
        # contiguously so DMA descriptors are 4KB DRAM runs
        return np.ascontiguousarray(
            w.reshape(KT, P, D_LOC).transpose(1, 0, 2).reshape(P, KT * D_LOC)
        ).astype(mmt)

    in_maps = []
    for c in range(N_CORES):
        b, r = divmod(c, 4)
        sl = slice(D_LOC * r, D_LOC * (r + 1))
        in_maps.append({
            "xT": xT[b],
            "wqT": pretile(WqT[:, sl]),
            "wkT": pretile(WkT[:, sl]),
            "wvT": pretile(WvT[:, sl]),
            "wpT": np.ascontiguousarray(WpT[sl, :]).astype(mmt),
            "bq": np.asarray(bq, f)[sl].reshape(D_LOC, 1).copy(),
            "bk": np.asarray(bk, f)[sl].reshape(D_LOC, 1).copy(),
        })
    return in_maps


def assemble_output(results, Wv, bv, Wp, bp):
    """Sum TP partials, transpose back, add folded biases."""
    f = np.float32
    bp_eff = np.asarray(bv, f) @ np.asarray(Wp, f).T + np.asarray(bp, f)
    out = np.empty((2, N_TOK, DIM), f)
    for b in range(2):
        acc = results[4 * b]["outT"].astype(f)
        for r in range(1, 4):
            acc = acc + results[4 * b + r]["outT"].astype(f)
        out[b] = acc.T + bp_eff
    return out


DT_MM_NAME = "bfloat16"

# Schraudolph-in-bf16 exp constants for the DVE-offloaded blocks:
# bf16 bits of 2^y ~= 128*y + 128*(127 - 0.043); with y = s*SCALE*log2(e)
# one DVE tensor_scalar (mult, add) writing int16 into the pt tile computes
# exp(s*SCALE) to ~2% RMS (common mode removed by the softmax normalize).
EXP_A = 128.0 * 1.4426950408889634 * SCALE
EXP_B = 16250.75


def kernel(x, Wq, bq, Wk, bk, Wv, bv, Wp, bp):
    from concourse.bass_utils import run_bass_kernel_spmd
    nc = _get_nc()
    in_maps = make_in_maps(x, Wq, bq, Wk, bk, Wv, bv, Wp, bp, DT_MM_NAME)
    res = run_bass_kernel_spmd(nc, in_maps, list(range(N_CORES)))
    return assemble_output(res.results, Wv, bv, Wp, bp)
